# revision 3
# baseline (speedup 1.0000x reference)
"""nn_AdditiveAttention_755914244534 — Trainium2 Bass kernel (8 cores).

Math: the reference's softmax runs over a trailing size-1 axis, so the
attention weights are exactly 1.0 and out[b, n, :] == values[b, 0, :] for
every n — independent of queries/keys/W_q/W_k/w_v. The kernel is a pure
broadcast of `values` (B, 1, DV) to (B, N, DV), bit-exact vs the reference.

Distribution: batch 32 is sharded 4-per-core across the 8 NeuronCores (pure
data parallel, no collectives). Each core materializes its (4, 4096, 512)
f32 shard = 32 MiB of HBM writes; the binding limit is the 16-port SBUF AXI
fabric (~436 GB/s -> ~27 GB/s per SDMA engine), so every byte of port
traffic besides the stores is shaved off.

Per-core schedule (store DMAs on the sync-engine HWDGE ring):
  1. load b0's value row broadcast into all 128 partitions (256 KiB),
  2. load b1-b3 rows to partition 0 only (6 KiB instead of 768 KiB of port
     traffic) — the idle TensorEngine fans them out to all 128 partitions
     via ones(1,128).T @ row(1,512) into PSUM (exact in f32, 1.0*x == x),
  3. "direct" store: batch 0's first 8 rows/partition straight from the
     loaded rows (2 KiB descriptors) — starts ~3 us into the block with no
     compute dependency,
  4. the Vector engine replicates each value row 8x within each partition
     (from SBUF for b0, straight from PSUM for b1-3) into tb tiles,
  5. the remaining 31.5 MiB streams from tb with 16 KiB contiguous
     descriptors at SDMA line rate.
Semaphores: separate sems per load (DMA completion order is not FIFO),
msem gates the ones-memset before PE, psem PE->DVE per batch (also keeps
PE writes and DVE reads on PSUM strictly ordered), vsem DVE->stores.
"""

import numpy as np

from concourse import bass, mybir
from concourse.bass_utils import run_bass_kernel_spmd

B, N, DV = 32, 4096, 512
NCORES = 8
BPC = B // NCORES  # 4 batches per core
P = 128
R = N // P  # 32 value-row copies per partition
K = 8  # replication factor inside SBUF (store descriptor = K*2 KiB)
R_DIRECT = 8  # rows per partition covered by the fast direct store (2 MiB)


def build_bass():
    nc = bass.Bass()
    vals = nc.declare_dram_parameter(
        "values", [BPC, DV], mybir.dt.float32, isOutput=False
    )
    out = nc.declare_dram_parameter(
        "out", [BPC, N, DV], mybir.dt.float32, isOutput=True
    )
    with (
        nc.sbuf_tensor([P, DV], mybir.dt.float32) as ts0,
        nc.sbuf_tensor([1, (BPC - 1) * DV], mybir.dt.float32) as tsm,
        nc.sbuf_tensor([1, P], mybir.dt.float32) as ones,
        nc.sbuf_tensor([P, BPC * K * DV], mybir.dt.float32) as tb,
        nc.psum_tensor([P, (BPC - 1) * DV], mybir.dt.float32) as ps,
        nc.semaphore("dma_sem") as sem,
        nc.semaphore("l0sem") as l0sem,
        nc.semaphore("lrsem") as lrsem,
        nc.semaphore("msem") as msem,
        nc.semaphore("psem") as psem,
        nc.semaphore("vsem") as vsem,
        nc.Block(no_gpsimd_drain=True) as block,
    ):

        @block.sync
        def _(sync):
            sync.dma_start(
                ts0[:].unsqueeze(1),
                vals[:1].unsqueeze(0).to_broadcast((P, 1, DV)),
            ).then_inc(l0sem, 16)
            sync.dma_start(
                tsm[:], vals[1:].rearrange("b d -> (b d)").unsqueeze(0)
            ).then_inc(lrsem, 16)
            sync.wait_ge(l0sem, 16)
            sync.dma_start(
                out[0].rearrange("(p r) d -> p r d", r=R)[:, :R_DIRECT],
                ts0[:].unsqueeze(1).to_broadcast((P, R_DIRECT, DV)),
            ).then_inc(sem, 16)
            sync.wait_ge(vsem, 1)
            sync.dma_start(
                out[0]
                .rearrange("(p r) d -> p r d", r=R)[:, R_DIRECT:]
                .rearrange("p (q e) d -> p q (e d)", e=K),
                tb[:, : K * DV]
                .unsqueeze(1)
                .to_broadcast((P, (R - R_DIRECT) // K, K * DV)),
            ).then_inc(sem, 16)
            for b in range(1, BPC):
                sync.wait_ge(vsem, b + 1)
                sync.dma_start(
                    out[b]
                    .rearrange("(p r) d -> p r d", r=R)
                    .rearrange("p (q e) d -> p q (e d)", e=K),
                    tb[:, b * K * DV : (b + 1) * K * DV]
                    .unsqueeze(1)
                    .to_broadcast((P, R // K, K * DV)),
                ).then_inc(sem, 16)
            sync.wait_ge(sem, 16 * (BPC + 1))
            sync.wait_ge(lrsem, 16)

        @block.tensor
        def _(tensor):
            tensor.wait_ge(msem, 1)
            tensor.wait_ge(lrsem, 16)
            for b in range(1, BPC):
                nc.tensor.matmul(
                    ps[:, (b - 1) * DV : b * DV],
                    ones[:],
                    tsm[:, (b - 1) * DV : b * DV],
                    start=True,
                    stop=True,
                ).then_inc(psem, 1)

        @block.vector
        def _(vector):
            vector.memset(ones[:], 1.0).then_inc(msem, 1)
            vector.wait_ge(l0sem, 16)
            vector.tensor_copy(
                tb[:, : K * DV].rearrange("p (r d) -> p r d", d=DV),
                ts0[:].unsqueeze(1).to_broadcast((P, K, DV)),
            ).then_inc(vsem, 1)
            for b in range(1, BPC):
                vector.wait_ge(psem, b)
                vector.tensor_copy(
                    tb[:, b * K * DV : (b + 1) * K * DV].rearrange(
                        "p (r d) -> p r d", d=DV
                    ),
                    ps[:, (b - 1) * DV : b * DV]
                    .unsqueeze(1)
                    .to_broadcast((P, K, DV)),
                ).then_inc(vsem, 1)
    return nc


def run(values: np.ndarray, trace: bool = False):
    """values: full (B, 1, DV) float32. Returns BassKernelResults."""
    nc = build_bass()
    v = np.ascontiguousarray(values, dtype=np.float32).reshape(B, DV)
    in_maps = [{"values": v[c * BPC : (c + 1) * BPC]} for c in range(NCORES)]
    return run_bass_kernel_spmd(
        nc, in_maps, core_ids=list(range(NCORES)), trace=trace
    )


def kernel(**inputs: np.ndarray) -> np.ndarray:
    res = run(inputs["values"], trace=False)
    return np.concatenate([r["out"] for r in res.results], axis=0)
